# revision 61
# baseline (speedup 1.0000x reference)
"""SNN 5-layer conv net (nn_Net_55405078118821) for 8 Trainium2 cores. v3.

Data-parallel over batch: each core processes 4 of 32 batch elements.

Per-core dataflow (all intermediates stay in SBUF):
  - Spike/input planes stored with padded EVEN row stride ST >= win+3 so
    every conv matmul streams ONE contiguous rhs slice (no per-t
    segmentation) and every scan/eviction op has even, 4B-aligned runs
    (enables DVE 2x perf modes). Garbage pad columns are never read by
    valid outputs.
  - conv as banded bf16 matmuls, one PSUM bank per 8-row output bank.
    L1 folds all 4 column taps into K (cin=3 -> K=96) with host-prepared
    shifted x planes; precision via (wterm,xplane) product pairs.
    L2-5 run nsp weight terms x 4 taps; the 3 spill rows from the next
    input bank use tap-pair-folded spill replicas (K=96) built on-device
    by SBUF->SBUF DMA.
  - LIF scan per timestep on DVE (f32 state, inverted spikes s'=1-s):
      u  = v - e            (tensor_tensor; e = -dv)
      s' = u is_lt vth      (tensor_scalar single-src -> 2x mode, bf16)
      v  = u * s'           (tensor_tensor reset)
  - Rolling batch schedule (no pair barrier): batches pipeline two-deep
    through the layers; next batch's L1 slots right after the previous
    pair's L4 so its x DMA and the L5 tails overlap; layer-5 spikes
    DMA'd out bf16; host computes means.
"""

import numpy as np
import ml_dtypes

import concourse.bass as bass
import concourse.bacc as bacc
import concourse.mybir as mybir
from concourse.tile import TileContext
from concourse.bass_utils import run_bass_kernel_spmd

N_CORES = 8
B_FULL, T = 32, 16
B_LOC = B_FULL // N_CORES
F32 = mybir.dt.float32
BF16 = mybir.dt.bfloat16
FP16 = mybir.dt.float16
NP_BF16 = ml_dtypes.bfloat16
NP_F16 = np.float16

# (Cin, Cout, Hin, Win) per layer; Hout = Hin-3, Wout = Win-3
LAYER_SHAPES = [(3, 16, 64, 64), (16, 16, 61, 61), (16, 16, 58, 58),
                (16, 16, 55, 55), (16, 6, 52, 52)]
ST = [64, 62, 58, 56, 52]           # input row stride per layer (even)
OST = [62, 58, 56, 52, 50]          # output tile row stride per layer (even)
# precision: L1 (wterm, xplane) product pairs; L2/L3 2x bf16 weight
# terms; L4/L5 single fp16 terms with fp16 scan state (validated in the
# numpy precision simulator: identical relmax to the all-bf16 config).
# plane-1 pair first so the next batch's plane-1 x DMA can start while
# plane-0 matmuls still run.
L1_PAIRS = [(0, 1), (0, 0), (1, 0)]
NP1 = len(L1_PAIRS)
NSP = [None, 2, 2, 1, 1]
W_NPDT = [None, NP_F16, NP_F16, NP_F16, NP_F16]     # weight term dtype
W_MYDT = [None, FP16, FP16, FP16, FP16]
S_MYDT = [FP16, FP16, FP16, FP16, FP16]             # spike tile view dtype
SCAN16 = [False, False, False, True, True]          # fp16 e/v scan


class LayerCfg:
    def __init__(self, idx, cin, cout, hin, win):
        self.idx = idx
        self.cin, self.cout, self.hin, self.win = cin, cout, hin, win
        self.hout, self.wout = hin - 3, win - 3
        self.nbk_in = (hin + 7) // 8
        self.nbk_out = (self.hout + 7) // 8
        self.st = ST[idx]
        self.ost = OST[idx]
        self.n = 8 * self.st - 3                     # matmul moving size
        self.mf = 8 * cout                           # full-block M
        self.p = min(self.mf, 128)
        self.nsp = NSP[idx]
        self.banks = []
        for q in range(self.nbk_out):
            r = min(8, self.hout - 8 * q)            # valid out rows
            r1 = min(8, hin - 8 * q)                 # in rows in bank q
            r2 = max(0, r - 5)                       # spill rows used
            self.banks.append((q, r, r1, r2))
        self.groups = [list(range(0, min(4, self.nbk_out))),
                       list(range(4, self.nbk_out))]


CFGS = [LayerCfg(i, *s) for i, s in enumerate(LAYER_SHAPES)]
L5 = CFGS[-1]
SOUT_FREE = L5.nbk_out * T * L5.ost                  # 7*16*50 = 5600


def _terms(a, n, dt=NP_BF16):
    a = np.asarray(a, np.float32)
    terms = []
    for _ in range(n):
        t = a.astype(dt).astype(np.float32)
        terms.append(t)
        a = a - t
    return terms


def _bf16_terms(a, n):
    return _terms(a, n, NP_BF16)


def _pack_A1(w):
    """L1 stationary: K=(rm,ci,dj)=96, M=(rho,co)=128, dj folded into K."""
    a = np.zeros((96, 128), np.float32)
    for rm in range(8):
        for ci in range(3):
            for dj in range(4):
                k = rm * 12 + ci * 4 + dj
                for rho in range(max(0, rm - 3), rm + 1):
                    a[k, rho * 16:(rho + 1) * 16] = w[:, ci, rm - rho, dj]
    return a


def _pack_B1(w):
    """L1 spill: K=(rm 0..2,ci,dj)=36, input row = 8+rm."""
    b = np.zeros((36, 128), np.float32)
    for rm in range(3):
        for ci in range(3):
            for dj in range(4):
                k = rm * 12 + ci * 4 + dj
                for rho in range(rm + 5, 8):
                    di = rm + 8 - rho
                    if 0 <= di <= 3:
                        b[k, rho * 16:(rho + 1) * 16] = w[:, ci, di, dj]
    return b


def _pack_A(w):
    """L2-5 stationary per term: K=(rm,ci)=8*cin, M=(rho,co), banded."""
    cout, cin = w.shape[0], w.shape[1]
    mf = 8 * cout
    a = np.zeros((8 * cin, 4 * mf), np.float32)
    for dj in range(4):
        for rm in range(8):
            for rho in range(max(0, rm - 3), rm + 1):
                a[rm * cin:(rm + 1) * cin,
                  dj * mf + rho * cout: dj * mf + (rho + 1) * cout] = \
                    w[:, :, rm - rho, dj].T
    return a


def _pack_B_direct(w):
    """L5 spill, unfolded per-tap: K=(rm,ci)=3*cin=48, 4 tap columns;
    rhs reads the next input bank's rows 0..2 directly from the spike
    tile (no DMA replicas)."""
    cout, cin = w.shape[0], w.shape[1]
    mf = 8 * cout
    b = np.zeros((3 * cin, 4 * mf), np.float32)
    for dj in range(4):
        for rm in range(3):
            for rho in range(rm + 5, 8):
                di = rm + 8 - rho
                if 0 <= di <= 3:
                    b[rm * cin:(rm + 1) * cin,
                      dj * mf + rho * cout: dj * mf + (rho + 1) * cout] = \
                        w[:, :, di, dj].T
    return b


def _pack_B(w, rmax=3):
    """L2-5 spill, tap-pair folded: K=(tap,rm,ci)=6*cin, two tap groups.
    rmax<3 zeroes spill rows rm>=rmax (for banks whose last input rows
    don't exist)."""
    cout, cin = w.shape[0], w.shape[1]
    mf = 8 * cout
    b = np.zeros((6 * cin, 2 * mf), np.float32)
    for g in range(2):
        for tap in range(2):
            for rm in range(rmax):
                dj = 2 * g + tap
                k0 = (tap * 3 + rm) * cin
                for rho in range(rm + 5, 8):
                    di = rm + 8 - rho
                    if 0 <= di <= 3:
                        b[k0:k0 + cin,
                          g * mf + rho * cout: g * mf + (rho + 1) * cout] = \
                            w[:, :, di, dj].T
    return b


def _pack_weights(inputs):
    m = {}
    terms = _terms(np.asarray(inputs["w1"], np.float32),
                   1 + max(wi for wi, _ in L1_PAIRS), NP_F16)
    m["wA1"] = np.concatenate(
        [_pack_A1(terms[wi]) for wi, _ in L1_PAIRS], axis=1).astype(NP_F16)
    m["wB1"] = np.concatenate(
        [_pack_B1(terms[wi]) for wi, _ in L1_PAIRS], axis=1).astype(NP_F16)
    for li in range(1, 5):
        cfg = CFGS[li]
        dt = W_NPDT[li]
        w = np.asarray(inputs[f"w{li + 1}"], np.float32)
        terms = _terms(w, cfg.nsp, dt)
        m[f"wA{li + 1}"] = np.concatenate(
            [_pack_A(-t) for t in terms], axis=1).astype(dt)
        packB = _pack_B_direct if li >= 3 else _pack_B
        m[f"wB{li + 1}"] = np.concatenate(
            [packB(-t) for t in terms], axis=1).astype(dt)
        if li == 2:
            # zero-padded variant for the partial-spill bank (L3 q=6)
            m[f"wBp{li + 1}"] = np.concatenate(
                [_pack_B(-t, rmax=2) for t in terms], axis=1).astype(dt)
    return m


def _pack_scalars(inputs):
    """Per-partition per-layer scalars: vthp = vth, cc = C, where C[co]
    is the quantized-weight kernel sum (conv(ones)); C=0 for layer 1."""
    s1 = np.zeros((128, 5), np.float32)
    s2 = np.zeros((128, 5), np.float32)
    for li, cfg in enumerate(CFGS):
        v = np.asarray(inputs[f"vth{li + 1}"], np.float32).reshape(-1)
        if li == 0:
            c = np.zeros(cfg.cout, np.float32)
        else:
            w = np.asarray(inputs[f"w{li + 1}"], np.float32)
            terms = _terms(w, cfg.nsp, W_NPDT[li])
            c = sum(t.sum(axis=(1, 2, 3)) for t in terms).astype(np.float32)
        for p in range(cfg.p):
            s1[p, li] = v[p % cfg.cout]
            s2[p, li] = -c[p % cfg.cout]
    return s1, s2


def _arrange_x(x):
    """[b,T,3,64,64] -> bf16 [b, 2, 96=(rm*12+ci*4+dj), (q*16+t)*64+w],
    value = xplane[b,t,ci,8q+rm,w+dj] (zero beyond column 63)."""
    bl = x.shape[0]
    planes = _terms(x, 2, NP_F16)
    out = np.zeros((bl, 2, 8, 3, 4, 8, T, 64), NP_F16)   # b pl rm ci dj q t w
    for pl in range(2):
        src = planes[pl].reshape(bl, T, 3, 8, 8, 64)     # b t ci q rm w
        src = src.transpose(0, 4, 2, 3, 1, 5)            # b rm ci q t w
        for dj in range(4):
            out[:, pl, :, :, dj, :, :, :64 - dj] = src[..., dj:]
    out = out.reshape(bl, 2, 96, 8 * T * 64)
    return np.ascontiguousarray(out)


_PROGRAM_CACHE = {}

# rolling slot order (b, li): two batches pipeline per layer, and the
# next pair's L1 slots are pulled forward (L5 spikes live in an e-pool
# tile, not the lp0 spike tile, so L1(b+2) no longer waits for L5(b)).
# Every f32 LIF scan (L1-L3) gets >=2 following PE slots to hide its
# serial chain + spill DMA.
SLOT_SEQ = [(0, 0), (1, 0), (0, 1), (1, 1), (0, 2), (1, 2), (0, 3), (2, 0),
            (1, 3), (3, 0), (0, 4), (2, 1), (1, 4), (3, 1), (2, 2), (3, 2),
            (2, 3), (3, 3), (2, 4), (3, 4)]


def _build_program():
    if "nc" in _PROGRAM_CACHE:
        return _PROGRAM_CACHE["nc"]
    nc = bacc.Bacc("TRN2", target_bir_lowering=False, debug=False)

    x_d = nc.dram_tensor("xr", [B_LOC, 2, 96, 8 * T * 64], FP16,
                         kind="ExternalInput").ap()
    wa_d = {0: nc.dram_tensor("wA1", [96, NP1 * 128], FP16,
                              kind="ExternalInput").ap()}
    wb_d = {0: nc.dram_tensor("wB1", [36, NP1 * 128], FP16,
                              kind="ExternalInput").ap()}
    wbp_d = {}
    for li in range(1, 5):
        cfg = CFGS[li]
        wa_d[li] = nc.dram_tensor(f"wA{li + 1}",
                                  [128, cfg.nsp * 4 * cfg.mf], W_MYDT[li],
                                  kind="ExternalInput").ap()
        wb_shape = [48, 4 * cfg.mf] if li >= 3 \
            else [96, cfg.nsp * 2 * cfg.mf]
        wb_d[li] = nc.dram_tensor(f"wB{li + 1}", wb_shape, W_MYDT[li],
                                  kind="ExternalInput").ap()
        if li == 2:
            wbp_d[li] = nc.dram_tensor(f"wBp{li + 1}",
                                       [96, cfg.nsp * 2 * cfg.mf],
                                       W_MYDT[li],
                                       kind="ExternalInput").ap()
    vthp_d = nc.dram_tensor("vthp", [128, 5], F32,
                            kind="ExternalInput").ap()
    cc_d = nc.dram_tensor("cc", [128, 5], F32,
                          kind="ExternalInput").ap()
    sout_d = nc.dram_tensor("sout", [B_LOC, 48, SOUT_FREE], FP16,
                            kind="ExternalOutput").ap()

    s_size = {0: max(CFGS[0].nbk_out * T * CFGS[0].ost,
                     CFGS[2].nbk_out * T * CFGS[2].ost,
                     CFGS[4].nbk_out * T * CFGS[4].ost),
              1: max(CFGS[1].nbk_out * T * CFGS[1].ost,
                     CFGS[3].nbk_out * T * CFGS[3].ost)}
    f_size = max((CFGS[li].nbk_in - 1) * T * CFGS[li].st
                 for li in range(1, 5))
    e_size = max(c.nbk_out * 8 * c.ost for c in CFGS[:3])
    e45_size = max(c.nbk_out * 8 * c.ost for c in CFGS[3:])

    with TileContext(nc) as tc:
        with (
            tc.tile_pool(name="wts", bufs=1) as wts,
            tc.tile_pool(name="xin", bufs=1) as xpool,
            tc.tile_pool(name="spk", bufs=1) as spool,
            tc.tile_pool(name="spill", bufs=1) as fpool,
            tc.tile_pool(name="scan", bufs=1) as upool,
            tc.tile_pool(name="ev", bufs=2) as epool,
            tc.tile_pool(name="psum", bufs=2, space="PSUM") as ppool,
        ):
            # --- constants (L1 weights + scalars + x(b0) first so the
            # first matmuls start as early as possible) ---
            wa_t, wb_t, wbp_t = {}, {}, {}
            wa_t[0] = wts.tile([96, NP1 * 128], FP16, tag="wa0", name="wa0")
            nc.sync.dma_start(out=wa_t[0][:, :], in_=wa_d[0])
            wb_t[0] = wts.tile([36, NP1 * 128], FP16, tag="wb0", name="wb0")
            nc.sync.dma_start(out=wb_t[0][:, :], in_=wb_d[0])
            vthp_t = wts.tile([128, 5], F32, tag="vthp")
            nc.sync.dma_start(out=vthp_t[:, :], in_=vthp_d)
            cc_t = wts.tile([128, 5], F32, tag="cc")
            nc.sync.dma_start(out=cc_t[:, :], in_=cc_d)
            x_t = {}
            for pl in range(2):
                x_t[pl] = xpool.tile([96, 8192], FP16,
                                     tag=f"x{pl}", name=f"x{pl}")

            def emit_x_dma(b):
                for pl in (1, 0):
                    nc.sync.dma_start(out=x_t[pl][:, :], in_=x_d[b, pl])

            # batch 0's x is chunked: nothing else is running yet, so the
            # first bank-group's matmuls can start ~6us earlier
            for c0, c1 in ((0, 5120), (5120, 8192)):
                for pl in (1, 0):
                    nc.sync.dma_start(out=x_t[pl][:, c0:c1],
                                      in_=x_d[0, pl, :, c0:c1])

            # HAM warm-up: tiny dummy matmuls spanning the initial x-DMA
            # wait keep the PE activity monitor busy so the real matmuls
            # start at the full 2.4 GHz clock instead of 1.2 GHz
            warm_ps = ppool.tile([128, 2048], F32, tag="ps", name="warmup")
            for _ in range(500):
                nc.tensor.matmul(warm_ps[0:5, 0:5], vthp_t[:, :],
                                 vthp_t[:, :], start=True, stop=True)
            for li in range(1, 5):
                cfg = CFGS[li]
                wa_t[li] = wts.tile([128, cfg.nsp * 4 * cfg.mf], W_MYDT[li],
                                    tag=f"wa{li}", name=f"wa{li}")
                nc.scalar.dma_start(out=wa_t[li][:, :], in_=wa_d[li])
                wb_shape = [48, 4 * cfg.mf] if li >= 3 \
                    else [96, cfg.nsp * 2 * cfg.mf]
                wb_t[li] = wts.tile(wb_shape, W_MYDT[li],
                                    tag=f"wb{li}", name=f"wb{li}")
                nc.scalar.dma_start(out=wb_t[li][:, :], in_=wb_d[li])
                if li == 2:
                    wbp_t[li] = wts.tile([96, cfg.nsp * 2 * cfg.mf],
                                         W_MYDT[li],
                                         tag=f"wbp{li}", name=f"wbp{li}")
                    nc.scalar.dma_start(out=wbp_t[li][:, :], in_=wbp_d[li])

            # spike tiles tags (b%2, li%2), bf16-declared with fp16 bitcast
            # views for L3-L5 data; spill [96, f_size] tags (b%2, tapgrp);
            # v state [128, 512] tags (b%2) x {f32, fp16}
            s_t, f_t, w_t, w16_t, l5out = {}, {}, {}, {}, {}
            for bp in range(2):
                for lp in range(2):
                    s_t[(bp, lp)] = spool.tile([128, s_size[lp]], FP16,
                                               tag=f"s{bp}{lp}",
                                               name=f"s{bp}{lp}")
                for g in range(2):
                    f_t[(bp, g)] = fpool.tile([96, f_size], FP16,
                                              tag=f"f{bp}{g}",
                                              name=f"f{bp}{g}")
                w_t[bp] = upool.tile(
                    [128, 512], F32, tag=f"w{bp}", name=f"w{bp}")
                w16_t[bp] = upool.tile(
                    [128, 512], FP16, tag=f"w16{bp}", name=f"w16{bp}")

            def emit_spill(b, li):
                """Spill replicas for layer li>=1 from layer li-1 spikes:
                f[(tap*3+rm)*16+ci, q, t, w] = s[rm*16+ci, q+1, t, w+2g+tap].
                One contiguous 48-partition DMA per (g, tap)."""
                bp = b % 2
                cfg = CFGS[li]
                st, nq = cfg.st, cfg.nbk_in - 1
                src_t = s_t[(bp, (li - 1) % 2)][:, :]
                src_v = src_t[:, 0:CFGS[li - 1].nbk_out * T * st].rearrange(
                    "p (q t w) -> p q t w", t=T, w=st)
                for g in range(2):
                    dst_t = f_t[(bp, g)][:, :]
                    dst_v = dst_t[:, 0:nq * T * st].rearrange(
                        "p (q t w) -> p q t w", t=T, w=st)
                    for tap in range(2):
                        sh = 2 * g + tap
                        dst = dst_v[tap * 48:tap * 48 + 48, :, :, 0:st - sh]
                        src = src_v[0:48, 1:nq + 1, :, sh:st]
                        nc.gpsimd.dma_start(out=dst, in_=src)

            def emit_layer(b, li):
                bp = b % 2
                cfg = CFGS[li]
                p, mf, n, st, ost = cfg.p, cfg.mf, cfg.n, cfg.st, cfg.ost
                ipl = 16 * st                          # input q-plane size
                nbt = cfg.nbk_out
                s_in = None if li == 0 else s_t[(bp, (li - 1) % 2)][:, :]
                if li == 4:
                    # L5 spikes land in an e-pool tile (fp16); this frees
                    # the lp0 spike tile so the next pair's L1 can overlap
                    l5_t = epool.tile([128, e_size], F32, tag="e",
                                      name=f"s5b{b}")
                    l5out[b] = l5_t
                    s_out = l5_t[:, :].bitcast(FP16)
                else:
                    s_out = s_t[(bp, li % 2)][:, :]
                sov = s_out[:, 0:nbt * T * ost].rearrange(
                    "p (q t w) -> p q t w", t=T, w=ost)
                vth_ap = vthp_t[0:p, li:li + 1]
                c_ap = cc_t[0:p, li:li + 1]
                # contiguous per-bank state: [p, nbt, ost] with k-stride=ost
                wtile = w16_t[bp] if SCAN16[li] else w_t[bp]
                vv = wtile[:, 0:nbt * ost].rearrange(
                    "p (k w) -> p k w", w=ost)[0:p]

                for h in range(2):
                    base_h = h * 8 * st
                    # e tile laid out (t, k, w) so each timestep's slice is
                    # one contiguous [p, nbt, ost] block; fp16 tag for the
                    # 16-bit scan layers
                    if SCAN16[li]:
                        et = epool.tile([128, e45_size], FP16, tag="e45",
                                        name=f"e{b}l{li}h{h}")
                    else:
                        et = epool.tile([128, e_size], F32, tag="e",
                                        name=f"e{b}l{li}h{h}")
                    ev4 = et[0:p, 0:nbt * 8 * ost].rearrange(
                        "p (t k w) -> p t k w", k=nbt, w=ost)
                    ev4kt = et[0:p, 0:nbt * 8 * ost].rearrange(
                        "p (t k w) -> p k t w", k=nbt, w=ost)
                    for gi, qs in enumerate(cfg.groups):
                        nbkg = len(qs)
                        ps = ppool.tile([128, 2048], F32, tag="ps",
                                        name=f"ps{b}l{li}h{h}g{gi}")
                        ps_f = ps[:, :]
                        nmm = {}
                        for bi, q in enumerate(qs):
                            _, r, r1, r2 = cfg.banks[q]
                            na = NP1 if li == 0 else cfg.nsp * 4
                            if r2 == 0:
                                nb = 0
                            elif li == 0:
                                nb = NP1
                            elif li >= 3:
                                nb = 4
                            else:
                                nb = cfg.nsp * 2
                            nmm[bi] = [na + nb, 0]

                        def mm(bi, lhs, rhs):
                            tot, done = nmm[bi]
                            out_ap = ps_f[0:p, bi * 512: bi * 512 + n]
                            nc.tensor.matmul(out_ap, lhs, rhs,
                                             start=(done == 0),
                                             stop=(done == tot - 1))
                            nmm[bi][1] += 1

                        if li == 0:
                            for pi, (wi, xi) in enumerate(L1_PAIRS):
                                lhs = wa_t[0][0:96, pi * 128:(pi + 1) * 128]
                                xt = x_t[xi][:, :]
                                for bi, q in enumerate(qs):
                                    rhs = xt[0:96, q * 1024 + base_h:
                                             q * 1024 + base_h + n]
                                    mm(bi, lhs, rhs)
                            for pi, (wi, xi) in enumerate(L1_PAIRS):
                                lhs = wb_t[0][0:36, pi * 128:(pi + 1) * 128]
                                xt = x_t[xi][:, :]
                                for bi, q in enumerate(qs):
                                    if cfg.banks[q][3] > 0:
                                        rhs = xt[0:36,
                                                 (q + 1) * 1024 + base_h:
                                                 (q + 1) * 1024 + base_h + n]
                                        mm(bi, lhs, rhs)
                        else:
                            s_in_f = s_in
                            for sp in range(cfg.nsp):
                                for dj in range(4):
                                    c0 = (sp * 4 + dj) * mf
                                    for bi, q in enumerate(qs):
                                        k1 = cfg.banks[q][2] * 16
                                        lhs = wa_t[li][0:k1, c0:c0 + mf]
                                        rhs = s_in_f[0:k1,
                                                     q * ipl + base_h + dj:
                                                     q * ipl + base_h + dj + n]
                                        mm(bi, lhs, rhs)
                            if li >= 3:
                                # unfolded per-tap spill, reading the next
                                # input bank's rows 0..2 directly
                                for dj in range(4):
                                    lhs = wb_t[li][0:48,
                                                   dj * mf:(dj + 1) * mf]
                                    for bi, q in enumerate(qs):
                                        if cfg.banks[q][3] > 0:
                                            o0 = (q + 1) * ipl + base_h + dj
                                            rhs = s_in_f[0:48, o0:o0 + n]
                                            mm(bi, lhs, rhs)
                            else:
                                for sp in range(cfg.nsp):
                                    for g in range(2):
                                        c0 = (sp * 2 + g) * mf
                                        ft = f_t[(bp, g)][:, :]
                                        for bi, q in enumerate(qs):
                                            r2 = cfg.banks[q][3]
                                            if r2 > 0:
                                                wbt = wb_t[li] if r2 == 3 \
                                                    else wbp_t[li]
                                                lhs = wbt[0:96, c0:c0 + mf]
                                                rhs = ft[0:96,
                                                         q * ipl + base_h:
                                                         q * ipl + base_h + n]
                                                mm(bi, lhs, rhs)

                        # --- evict psum to SBUF on the ACT engine, negated
                        # with the conv(ones) constant folded: e = -(q+C)
                        # = -dv. Width = ost (even, covers wout). ---
                        ps_v = ps_f.rearrange("p (k f) -> p k f", k=4)
                        ps4 = ps_v[0:p, 0:nbkg, 0:8 * st].rearrange(
                            "p k (t w) -> p k t w", w=st)[:, :, :, 0:ost]
                        epart = ev4kt[:, qs[0]:qs[0] + nbkg, :, :]
                        nc.scalar.activation(
                            epart, ps4, mybir.ActivationFunctionType.Identity,
                            bias=c_ap, scale=-1.0)

                    # --- LIF scan (all DVE, f32 state, inverted spikes);
                    # u overwrites the (dead) e slot in place:
                    #   e[t] <- v - e[t] = u      (e = -dv)
                    #   s'   = u is_lt vth        (tensor_scalar, 2x mode)
                    #   v    = u * s'
                    for t in range(8):
                        tt = h * 8 + t
                        ev = ev4[:, t, :, :]
                        sw = sov[0:p, 0:nbt, tt, 0:ost]
                        if tt == 0:
                            nc.vector.tensor_scalar(
                                ev, ev, -1.0, None, mybir.AluOpType.mult)
                        else:
                            nc.vector.tensor_tensor(
                                out=ev, in0=vv, in1=ev,
                                op=mybir.AluOpType.subtract)
                        nc.vector.tensor_scalar(
                            sw, ev, vth_ap, None, mybir.AluOpType.is_lt)
                        nc.vector.tensor_tensor(
                            out=vv, in0=ev, in1=sw,
                            op=mybir.AluOpType.mult)

            def emit_sout(b):
                nc.gpsimd.dma_start(
                    out=sout_d[b],
                    in_=l5out[b][:, :].bitcast(FP16)[0:48, 0:SOUT_FREE])

            for b, li in SLOT_SEQ:
                emit_layer(b, li)
                if li < 2:
                    # L4/L5 need no replicas (direct spill reads): only
                    # L2/L3 consume the f tiles
                    emit_spill(b, li + 1)
                elif li == 4:
                    emit_sout(b)
                if li == 0 and b + 1 < B_LOC:
                    emit_x_dma(b + 1)

    nc.compile()
    _PROGRAM_CACHE["nc"] = nc
    return nc


def _host_inputs(inputs):
    m = _pack_weights(inputs)
    s1, s2 = _pack_scalars(inputs)
    m["vthp"] = s1
    m["cc"] = s2
    return m


def decode_sout(sout):
    """[B_LOC, 48, SOUT_FREE] fp16 -> [B_LOC, T, 6] spike means."""
    a = 1.0 - np.asarray(sout, np.float32).reshape(B_LOC, 8, 6, L5.nbk_out,
                                                   T, L5.ost)
    rho = np.arange(8)[:, None]
    qq = np.arange(L5.nbk_out)[None, :]
    mask = (8 * qq + rho) < L5.hout                     # [rho, q]
    a = a.transpose(0, 4, 2, 1, 3, 5)                   # [b, t, c, rho, q, j]
    vals = a[:, :, :, mask, :][:, :, :, :, :L5.wout]    # [b, t, c, 49, 49]
    return vals.mean(axis=(3, 4)).astype(np.float32)


def run_spmd(inputs, **kw):
    nc = _build_program()
    x = np.asarray(inputs["x"], np.float32)
    const = _host_inputs(inputs)
    in_maps = []
    for c in range(N_CORES):
        m = dict(const)
        m["xr"] = _arrange_x(x[c * B_LOC:(c + 1) * B_LOC])
        in_maps.append(m)
    return run_bass_kernel_spmd(nc, in_maps, list(range(N_CORES)), **kw)


def kernel(**inputs):
    res = run_spmd(inputs)
    outs = [decode_sout(r["sout"]) for r in res.results]
    return np.concatenate(outs, axis=0)
